# revision 20
# baseline (speedup 1.0000x reference)
"""Masked attention for (B=8, S=2048, E=A=256), f32 in/out.

Sharding: data-parallel over batch B across the 8 NeuronCores (one batch
element per core, no collectives).

Per-core dataflow (all on-chip after the input DMAs):
  xT[E,S] -> q8,k8 ([128, a-chunk=2, S] fp8e4; bias added on evacuation)
          -> v [S, A+2] fp16 (bias via K=1 ones-row matmul; cols A,A+1 hold
             1.0 so the PV matmul also produces the softmax denominator)
  scores TRANSPOSED, fp8 DoubleRow (contraction A=256 in one matmul):
    scT[sk_chunk=128p, sq 512] = k8_chunk.T @ q8  (psum tile [128, 1024])
  attnT = exp(scT/16) * maskT  (no max subtraction: |scores/16| < ~3)
  outP[sq=128p, A+2] += attnT_chunk.T @ v_chunk   (accumulate over sk, fp16)
  out rows = [num | den] in bf16; the final num/den divide runs on host.

Schedule notes:
 - junk matmuls (no data deps) run from the first post-preamble cycle so the
   PE HAM clock gate reaches 8/8 before the projections arrive.
 - x arrives as ONE big DMA per half (DMA trigger instructions cost ~0.65us
   of queue time each, so many small DMAs starve the PE at the head); all
   weights ride one packed tensor; mask DMAs are gated behind x so they do
   not steal HBM bandwidth from the critical-path loads.
 - the attention loop is software-pipelined one group ahead (scores for
   group t+1 are queued before the PV matmuls of group t) so the exp+mask
   latency stays hidden across j-block boundaries.
"""

import sys

sys.path.insert(0, "/opt/trn_rl_repo")

import numpy as np
import ml_dtypes

B, S, E, A = 8, 2048, 256, 256
N_CORES = 8

SQBLK = 512                 # Sq rows per outer block
N_SQBLK = S // SQBLK        # 4
SQSUB = 128                 # Sq rows per PV psum tile
N_SQSUB = SQBLK // SQSUB    # 4
SKCH = 128                  # Sk rows per score chunk (psum partitions)
N_SKCH = S // SKCH          # 16
GRP = 2                     # sk chunks per scores psum tile ([128, GRP*SQBLK])
N_GRP = N_SKCH // GRP       # 8
MTILE = 4                   # sk chunks per mask sbuf tile
N_WARM = 16                 # junk warm-up matmuls (N=256 cold ~215ns each)
WCOL = 2 * A + (A + 2)      # packed weight columns: wq | wk | wv+ones-pad

SCALE = 1.0 / np.sqrt(np.float32(A))


def _emit(nc, tc, ctx, tensors):
    import concourse.bass as bass
    import concourse.mybir as mybir

    f32 = mybir.dt.float32
    f16 = mybir.dt.float16
    bf16 = mybir.dt.bfloat16
    f8 = mybir.dt.float8e4
    AF = mybir.ActivationFunctionType
    DR = mybir.MatmulPerfMode.DoubleRow

    xT, maskT, W_pack, bias_pack, row_pack, out = tensors

    consts = ctx.enter_context(tc.tile_pool(name="consts", bufs=1))
    big = ctx.enter_context(tc.tile_pool(name="big", bufs=1))
    mpool = ctx.enter_context(tc.tile_pool(name="mask", bufs=16))
    epool = ctx.enter_context(tc.tile_pool(name="exp", bufs=4))
    apool = ctx.enter_context(tc.tile_pool(name="attn", bufs=4))
    opool = ctx.enter_context(tc.tile_pool(name="outsb", bufs=6))
    spool = ctx.enter_context(tc.tile_pool(name="small", bufs=8))
    ps_sc = ctx.enter_context(tc.tile_pool(name="ps_sc", bufs=2, space="PSUM"))
    ps_sm = ctx.enter_context(tc.tile_pool(name="ps_sm", bufs=4, space="PSUM"))

    # ---- HAM warm-up: dense junk matmul stream (results never consumed) so
    # the PE clock gate is already 8/8 when the projections arrive ----
    warm_sb = consts.tile([128, 256], f16, tag="warm_sb")
    nc.gpsimd.memset(warm_sb, 1.0)
    warm_ps = ps_sc.tile([128, GRP * SQBLK], f32, name="warm_ps", tag="sc")
    for _ in range(N_WARM):
        nc.tensor.matmul(
            warm_ps[:, :256], lhsT=warm_sb[:, :128], rhs=warm_sb, start=True, stop=True
        )

    # ---- inputs. Load order puts every tensor on a ring slot that lands
    # just before its first use: bias + x j0/j1 on the sync ring, weights +
    # row + x j2/j3 on the scalar ring (ring-FIFO position protects them
    # from the mask traffic) ----
    bias_sb = consts.tile([128, 4], f32, tag="bias_pack")
    nc.sync.dma_start(out=bias_sb, in_=bias_pack)
    wpack_sb = consts.tile([128, 2, WCOL], f16, tag="wpack")
    nc.scalar.dma_start(out=wpack_sb, in_=W_pack)
    row_sb = consts.tile([1, A + 2 + 128], f16, tag="row_pack")
    nc.scalar.dma_start(out=row_sb, in_=row_pack)
    xq_sb = []
    for j in range(N_SQBLK):
        t = big.tile([128, 2, SQBLK], f16, name=f"xq{j}", tag=f"xq{j}")
        (nc.sync if j < 2 else nc.scalar).dma_start(out=t, in_=xT[j])
        xq_sb.append(t)
    Wq_sb = [wpack_sb[:, e, 0:A] for e in range(2)]
    Wk_sb = [wpack_sb[:, e, A : 2 * A] for e in range(2)]
    Wv_sb = [wpack_sb[:, e, 2 * A : WCOL] for e in range(2)]
    bq_sb = [bias_sb[:, 0:1], bias_sb[:, 1:2]]
    bk_sb = [bias_sb[:, 2:3], bias_sb[:, 3:4]]
    bv_sb = row_sb[:, : A + 2]
    ones_sb = row_sb[:, A + 2 :]

    def x_rhs(e, j):  # [128, 512] moving operand for the qk projections
        return xq_sb[j][:, e, :]

    def x_lhsT(e, c):  # [128, 128] stationary operand for the v projection
        return xq_sb[c // 4][:, e, bass.ts(c % 4, 128)]

    # ---- mask DMAs: gpsimd ring ----
    mask_tiles = {}
    for j in range(N_SQBLK):
        for t in range(N_SKCH // MTILE):
            mt = mpool.tile([128, MTILE, SQBLK], f16, name=f"mask{j}_{t}", tag="mask")
            nc.gpsimd.dma_start(out=mt, in_=maskT[j][:, bass.ts(t, MTILE), :])
            mask_tiles[(j, t)] = mt

    # ---- projections ----
    # q8/k8: [128, a-chunk, S] fp8e4 (DoubleRow layout: partition = a%128,
    # middle dim = a-chunk, free = sequence). psum per (a, Sq512), E-chunk
    # accumulated; bias added + fp8 quantized on the DVE evacuation.
    q8_sb = big.tile([128, 2, S], f8, tag="q8")
    k8_sb = big.tile([128, 2, S], f8, tag="k8")
    v_sb = [None] * N_SKCH
    # per-j steps so the projections start as soon as x's first quarter
    # lands; step s also produces v chunk s
    for s in range(16):
        j, a, wi = s // 4, (s % 4) // 2, s % 2
        W_sb, b_sb, dst = (
            (Wq_sb, bq_sb[a], q8_sb),
            (Wk_sb, bk_sb[a], k8_sb),
        )[wi]
        pp = ps_sm.tile([128, 512], f32, name=f"pp{s}", tag="ps")
        for e in range(2):
            nc.tensor.matmul(
                pp,
                lhsT=W_sb[e][:, bass.ts(a, 128)],
                rhs=x_rhs(e, j),
                start=(e == 0),
                stop=(e == 1),
            )
        vp = ps_sm.tile([128, 512], f32, name=f"vp{s}", tag="ps")
        for e in range(2):
            nc.tensor.matmul(
                vp[:, : A + 2],
                lhsT=x_lhsT(e, s),
                rhs=Wv_sb[e],
                start=(e == 0),
                stop=False,
            )
        nc.tensor.matmul(
            vp[:, : A + 2],
            lhsT=ones_sb,
            rhs=bv_sb,
            start=False,
            stop=True,
        )
        nc.vector.tensor_scalar_add(dst[:, a, bass.ts(j, SQBLK)], pp, b_sb)
        vt = big.tile([128, A + 2], f16, tag=f"v{s}", name=f"v{s}")
        nc.scalar.copy(vt, vp[:, : A + 2])
        v_sb[s] = vt

    # ---- attention: flat pipeline over (j, g) groups, scores one group
    # ahead of the PV matmuls so exp+mask latency hides at j boundaries ----
    groups = [(j, g) for j in range(N_SQBLK) for g in range(N_GRP)]
    LA = 1
    at_tiles = {}
    out_ps_by_j = {}

    def emit_scores(t):
        j, g = groups[t]
        js = bass.ts(j, SQBLK)
        sc = ps_sc.tile([128, GRP * SQBLK], f32, tag="sc")
        for c in range(GRP):
            ch = g * GRP + c
            nc.tensor.matmul(
                sc[:, bass.ts(c, SQBLK)],
                lhsT=k8_sb[:, :, bass.ts(ch, 128)],
                rhs=q8_sb[:, :, js],
                start=True,
                stop=True,
                perf_mode=DR,
            )
        ex = epool.tile([128, GRP * SQBLK], f16)
        nc.scalar.activation(ex, sc, AF.Exp, bias=0.0, scale=float(SCALE))
        at = apool.tile([128, GRP, SQBLK], f16)
        mslice = mask_tiles[(j, (g * GRP) // MTILE)][
            :, bass.ds((g * GRP) % MTILE, GRP), :
        ]
        nc.vector.tensor_mul(at, ex.rearrange("p (c s) -> p c s", c=GRP), mslice)
        at_tiles[t] = at

    def emit_pv(t):
        j, g = groups[t]
        if g == 0:
            out_ps_by_j[j] = [
                ps_sm.tile([128, 512], f32, name=f"out_ps{j}_{s}", tag="ps")
                for s in range(N_SQSUB)
            ]
        at = at_tiles.pop(t)
        for c in range(GRP):
            ch = g * GRP + c
            for sq in range(N_SQSUB):
                nc.tensor.matmul(
                    out_ps_by_j[j][sq][:, : A + 2],
                    lhsT=at[:, c, bass.ts(sq, SQSUB)],
                    rhs=v_sb[ch],
                    start=(ch == 0),
                    stop=(ch == N_SKCH - 1),
                )

    def emit_evac(j):
        # raw [num | den] rows to HBM in bf16 via ONE packed tile + one DMA
        # per j-block (host does the divide). Copies split across ACT and
        # DVE so neither queue stalls the next group.
        ob = opool.tile([128, N_SQSUB * (A + 2)], bf16)
        for sq in range(N_SQSUB):
            src = out_ps_by_j[j][sq][:, : A + 2]
            dst = ob[:, bass.ts(sq, A + 2)]
            if sq < 2:
                nc.scalar.copy(dst, src)
            else:
                nc.vector.tensor_copy(dst, src)
        nc.sync.dma_start(out=out[j], in_=ob)

    n = len(groups)
    # last two score groups are hoisted ahead of their PV batches (deeper
    # lookahead at the stream tail) so the final exp+mask latency hides
    # under the remaining PV work instead of draining exposed
    order = []
    for t in range(n - 2):
        order.append(("sc", t))
        if t >= LA:
            order.append(("pv", t - LA))
    order += [("sc", n - 2), ("sc", n - 1)]
    order += [("pv", t) for t in range(n - 2 - LA, n)]
    for kind, t in order:
        if kind == "sc":
            emit_scores(t)
        else:
            emit_pv(t)
            jd, gd = groups[t]
            if gd == N_GRP - 1:
                emit_evac(jd)


def build_nc():
    from contextlib import ExitStack

    import concourse.bacc as bacc
    import concourse.tile as tile
    import concourse.mybir as mybir

    f32 = mybir.dt.float32
    f16 = mybir.dt.float16
    bf16 = mybir.dt.bfloat16

    nc = bacc.Bacc("TRN2", target_bir_lowering=False, debug=False)
    xT = nc.dram_tensor("xT", [N_SQBLK, 128, 2, SQBLK], f16, kind="ExternalInput").ap()
    maskT = nc.dram_tensor(
        "maskT", [N_SQBLK, 128, N_SKCH, SQBLK], f16, kind="ExternalInput"
    ).ap()
    W_pack = nc.dram_tensor("W_pack", [128, 2, WCOL], f16, kind="ExternalInput").ap()
    bias_pack = nc.dram_tensor("bias_pack", [128, 4], f32, kind="ExternalInput").ap()
    row_pack = nc.dram_tensor(
        "row_pack", [1, A + 2 + 128], f16, kind="ExternalInput"
    ).ap()
    out = nc.dram_tensor(
        "out", [N_SQBLK, 128, N_SQSUB * (A + 2)], bf16, kind="ExternalOutput"
    ).ap()

    tensors = (xT, maskT, W_pack, bias_pack, row_pack, out)
    with tile.TileContext(nc) as tc:
        with ExitStack() as ctx:
            _emit(nc, tc, ctx, tensors)
    nc.compile()
    return nc


def pack_inputs(x, mask, Wq, bq, Wk, bk, Wv, bv):
    """Host-side packing: per-core input maps (core c <- batch c)."""
    hdt = np.float16
    x = np.asarray(x, dtype=np.float32)
    mask = np.asarray(mask)
    # maskT[b, j, p, c, s] = mask[b, j*512+s, c*128+p], as {0.0, 1.0}
    from concurrent.futures import ThreadPoolExecutor

    def _pack_mask(b):
        return np.ascontiguousarray(
            mask[b]
            .transpose(1, 0)
            .reshape(N_SKCH, 128, N_SQBLK, SQBLK)
            .transpose(2, 1, 0, 3)
            .astype(hdt)
        )

    with ThreadPoolExecutor(max_workers=8) as tp:
        mt = list(tp.map(_pack_mask, range(B)))
    # W_pack[p, e, :] = [wq | wk | wv+pad] for E-chunk e
    Wq = np.asarray(Wq, hdt).reshape(2, 128, A)
    Wk = np.asarray(Wk, hdt).reshape(2, 128, A)
    Wv = np.concatenate([np.asarray(Wv, hdt), np.zeros((E, 2), hdt)], axis=1).reshape(
        2, 128, A + 2
    )
    W_pack = np.ascontiguousarray(
        np.concatenate([Wq, Wk, Wv], axis=2).transpose(1, 0, 2)
    )
    bq = np.asarray(bq, np.float32).reshape(2, 128)
    bk = np.asarray(bk, np.float32).reshape(2, 128)
    bias_pack = np.ascontiguousarray(
        np.stack([bq[0], bq[1], bk[0], bk[1]], axis=1)
    )
    row_pack = np.concatenate(
        [
            np.asarray(bv, hdt).reshape(-1),
            np.ones(2, hdt),
            np.ones(128, hdt),
        ]
    ).reshape(1, A + 2 + 128)
    in_maps = []
    for b in range(N_CORES):
        # xT[j, p, e, s_in_block] from x[b] [S, E]
        xb = np.ascontiguousarray(
            x[b].T.astype(hdt).reshape(2, 128, N_SQBLK, SQBLK).transpose(2, 1, 0, 3)
        )
        in_maps.append(
            {
                "xT": xb,
                "maskT": mt[b],
                "W_pack": W_pack,
                "bias_pack": bias_pack,
                "row_pack": row_pack,
            }
        )
    return in_maps


_NC_CACHE = None


def _get_nc():
    global _NC_CACHE
    if _NC_CACHE is None:
        _NC_CACHE = build_nc()
    return _NC_CACHE


def unpack_out(raw):
    """[B, 4, 128, 4*(A+2)] bf16 raw [num|den] tiles -> [B, S, A] f32."""
    raw = np.asarray(raw).astype(np.float32)
    raw = raw.reshape(B, N_SQBLK, 128, N_SQSUB, A + 2).transpose(0, 1, 3, 2, 4)
    raw = raw.reshape(B, S, A + 2)
    return raw[:, :, :A] / raw[:, :, A : A + 1]


def kernel(x, mask, Wq, bq, Wk, bk, Wv, bv):
    from concourse.bass_utils import run_bass_kernel_spmd

    in_maps = pack_inputs(x, mask, Wq, bq, Wk, bk, Wv, bv)
    nc = _get_nc()
    res = run_bass_kernel_spmd(nc, in_maps, core_ids=list(range(N_CORES)))
    raw = np.stack(
        [np.asarray(res.results[c]["out"]) for c in range(N_CORES)], axis=0
    )
    return unpack_out(raw)


if __name__ == "__main__":
    nc = build_nc()
    n = sum(len(bb.instructions) for bb in nc.main_func.blocks)
    print("built ok; instructions:", n)


# revision 24
# speedup vs baseline: 1.1330x; 1.1330x over previous
"""Masked attention for (B=8, S=2048, E=A=256), f32 in/out.

Sharding: data-parallel over batch B across the 8 NeuronCores (one batch
element per core, no collectives).

Per-core dataflow (all on-chip after the input DMAs):
  xT[E,S] -> q8,k8 ([128, a-chunk=2, S] fp8e4; bias added on evacuation)
          -> v [S, A+2] fp16 (bias via K=1 ones-row matmul; cols A,A+1 hold
             1.0 so the PV matmul also produces the softmax denominator)
  scores TRANSPOSED, fp8 DoubleRow (contraction A=256 in one matmul):
    scT[sk_chunk=128p, sq 512] = k8_chunk.T @ q8  (psum tile [128, 1024])
  attnT = exp(scT/16) * maskT  (no max subtraction: |scores/16| < ~3)
  outP[sq=128p, A+2] += attnT_chunk.T @ v_chunk   (accumulate over sk, fp16)
  out rows = [num | den] in bf16; the final num/den divide runs on host.

Schedule notes:
 - junk matmuls (no data deps) run from the first post-preamble cycle so the
   PE HAM clock gate reaches 8/8 before the projections arrive.
 - x arrives as ONE big DMA per half (DMA trigger instructions cost ~0.65us
   of queue time each, so many small DMAs starve the PE at the head); all
   weights ride one packed tensor; mask DMAs are gated behind x so they do
   not steal HBM bandwidth from the critical-path loads.
 - the attention loop is software-pipelined one group ahead (scores for
   group t+1 are queued before the PV matmuls of group t) so the exp+mask
   latency stays hidden across j-block boundaries.
"""

import sys

sys.path.insert(0, "/opt/trn_rl_repo")

import numpy as np
import ml_dtypes

B, S, E, A = 8, 2048, 256, 256
N_CORES = 8

SQBLK = 512                 # Sq rows per outer block
N_SQBLK = S // SQBLK        # 4
SQSUB = 128                 # Sq rows per PV psum tile
N_SQSUB = SQBLK // SQSUB    # 4
SKCH = 128                  # Sk rows per score chunk (psum partitions)
N_SKCH = S // SKCH          # 16
GRP = 2                     # sk chunks per scores psum tile ([128, GRP*SQBLK])
N_GRP = N_SKCH // GRP       # 8
MTILE = 4                   # sk chunks per mask sbuf tile
N_WARM = 16                 # junk warm-up matmuls (N=256 cold ~215ns each)
WCOL = 2 * A + (A + 2)      # packed weight columns: wq | wk | wv+ones-pad

SCALE = 1.0 / np.sqrt(np.float32(A))


def _emit(nc, tc, ctx, tensors):
    import concourse.bass as bass
    import concourse.mybir as mybir

    f32 = mybir.dt.float32
    f16 = mybir.dt.float16
    bf16 = mybir.dt.bfloat16
    f8 = mybir.dt.float8e4
    AF = mybir.ActivationFunctionType
    DR = mybir.MatmulPerfMode.DoubleRow

    xT, maskT, W_pack, bias_pack, row_pack, out = tensors

    consts = ctx.enter_context(tc.tile_pool(name="consts", bufs=1))
    big = ctx.enter_context(tc.tile_pool(name="big", bufs=1))
    mpool = ctx.enter_context(tc.tile_pool(name="mask", bufs=16))
    epool = ctx.enter_context(tc.tile_pool(name="exp", bufs=4))
    apool = ctx.enter_context(tc.tile_pool(name="attn", bufs=4))
    opool = ctx.enter_context(tc.tile_pool(name="outsb", bufs=6))
    spool = ctx.enter_context(tc.tile_pool(name="small", bufs=8))
    ps_sc = ctx.enter_context(tc.tile_pool(name="ps_sc", bufs=2, space="PSUM"))
    ps_sm = ctx.enter_context(tc.tile_pool(name="ps_sm", bufs=4, space="PSUM"))

    # ---- HAM warm-up: dense junk matmul stream (results never consumed) so
    # the PE clock gate is already 8/8 when the projections arrive ----
    warm_sb = consts.tile([128, 256], f16, tag="warm_sb")
    nc.gpsimd.memset(warm_sb, 1.0)
    warm_ps = ps_sc.tile([128, GRP * SQBLK], f32, name="warm_ps", tag="sc")
    for _ in range(N_WARM):
        nc.tensor.matmul(
            warm_ps[:, :256], lhsT=warm_sb[:, :128], rhs=warm_sb, start=True, stop=True
        )

    # ---- inputs: weights first on the scalar HWDGE ring (their ring-FIFO
    # position guarantees they are not starved by the mask traffic), one big
    # DMA per x half, tiny bias/row DMAs ----
    wpack_sb = consts.tile([128, 2, WCOL], f16, tag="wpack")
    nc.scalar.dma_start(out=wpack_sb, in_=W_pack)
    xh_sb = []
    for h in range(2):
        t = big.tile([128, 2, 1024], f16, name=f"xh{h}", tag=f"xh{h}")
        (nc.sync if h == 0 else nc.scalar).dma_start(out=t, in_=xT[h])
        xh_sb.append(t)
    Wq_sb = [wpack_sb[:, e, 0:A] for e in range(2)]
    Wk_sb = [wpack_sb[:, e, A : 2 * A] for e in range(2)]
    Wv_sb = [wpack_sb[:, e, 2 * A : WCOL] for e in range(2)]

    bias_sb = consts.tile([128, 4], f32, tag="bias_pack")
    nc.sync.dma_start(out=bias_sb, in_=bias_pack)
    bq_sb = [bias_sb[:, 0:1], bias_sb[:, 1:2]]
    bk_sb = [bias_sb[:, 2:3], bias_sb[:, 3:4]]
    row_sb = consts.tile([1, A + 2 + 128], f16, tag="row_pack")
    nc.sync.dma_start(out=row_sb, in_=row_pack)
    bv_sb = row_sb[:, : A + 2]
    ones_sb = row_sb[:, A + 2 :]

    def x_rhs(e, j):  # [128, 512] moving operand for the qk projections
        return xh_sb[j // 2][:, e, bass.ts(j % 2, SQBLK)]

    def x_lhsT(e, c):  # [128, 128] stationary operand for the v projection
        j = c // 4
        return xh_sb[j // 2][:, e, bass.ds((j % 2) * SQBLK + (c % 4) * 128, 128)]

    # ---- mask DMAs: gpsimd ring ----
    mask_tiles = {}
    for j in range(N_SQBLK):
        for t in range(N_SKCH // MTILE):
            mt = mpool.tile([128, MTILE, SQBLK], f16, name=f"mask{j}_{t}", tag="mask")
            nc.gpsimd.dma_start(out=mt, in_=maskT[j][:, bass.ts(t, MTILE), :])
            mask_tiles[(j, t)] = mt

    # ---- projections ----
    # q8/k8: [128, a-chunk, S] fp8e4 (DoubleRow layout: partition = a%128,
    # middle dim = a-chunk, free = sequence). psum per (a, Sq512), E-chunk
    # accumulated; bias added + fp8 quantized on the DVE evacuation.
    q8_sb = big.tile([128, 2, S], f8, tag="q8")
    k8_sb = big.tile([128, 2, S], f8, tag="k8")
    v_sb = [None] * N_SKCH
    qk_steps = [
        (jp, a, wi)
        for jp in ((0, 1), (2, 3))
        for a in range(2)
        for wi in range(2)
    ]
    for s, (jp, a, wi) in enumerate(qk_steps):
        W_sb, b_sb, dst = (
            (Wq_sb, bq_sb[a], q8_sb),
            (Wk_sb, bk_sb[a], k8_sb),
        )[wi]
        pss = [
            ps_sm.tile([128, 512], f32, name=f"pp{s}_{j}", tag="ps") for j in jp
        ]
        for e in range(2):
            for i, j in enumerate(jp):
                nc.tensor.matmul(
                    pss[i],
                    lhsT=W_sb[e][:, bass.ts(a, 128)],
                    rhs=x_rhs(e, j),
                    start=(e == 0),
                    stop=(e == 1),
                )
        cpair = (2 * s, 2 * s + 1)
        vps = [
            ps_sm.tile([128, 512], f32, name=f"vp{c}", tag="ps") for c in cpair
        ]
        for e in range(2):
            for i, c in enumerate(cpair):
                nc.tensor.matmul(
                    vps[i][:, : A + 2],
                    lhsT=x_lhsT(e, c),
                    rhs=Wv_sb[e],
                    start=(e == 0),
                    stop=False,
                )
        for i, c in enumerate(cpair):
            nc.tensor.matmul(
                vps[i][:, : A + 2],
                lhsT=ones_sb,
                rhs=bv_sb,
                start=False,
                stop=True,
            )
        for i, j in enumerate(jp):
            nc.vector.tensor_scalar_add(dst[:, a, bass.ts(j, SQBLK)], pss[i], b_sb)
        for i, c in enumerate(cpair):
            vt = big.tile([128, A + 2], f16, tag=f"v{c}", name=f"v{c}")
            nc.scalar.copy(vt, vps[i][:, : A + 2])
            v_sb[c] = vt

    # ---- attention: flat pipeline over (j, g) groups, scores one group
    # ahead of the PV matmuls so exp+mask latency hides at j boundaries ----
    groups = [(j, g) for j in range(N_SQBLK) for g in range(N_GRP)]
    LA = 1
    at_tiles = {}
    out_ps_by_j = {}

    def emit_scores(t):
        j, g = groups[t]
        js = bass.ts(j, SQBLK)
        sc = ps_sc.tile([128, GRP * SQBLK], f32, tag="sc")
        for c in range(GRP):
            ch = g * GRP + c
            nc.tensor.matmul(
                sc[:, bass.ts(c, SQBLK)],
                lhsT=k8_sb[:, :, bass.ts(ch, 128)],
                rhs=q8_sb[:, :, js],
                start=True,
                stop=True,
                perf_mode=DR,
            )
        ex = epool.tile([128, GRP * SQBLK], f16)
        nc.scalar.activation(ex, sc, AF.Exp, bias=0.0, scale=float(SCALE))
        at = apool.tile([128, GRP, SQBLK], f16)
        mslice = mask_tiles[(j, (g * GRP) // MTILE)][
            :, bass.ds((g * GRP) % MTILE, GRP), :
        ]
        nc.vector.tensor_mul(at, ex.rearrange("p (c s) -> p c s", c=GRP), mslice)
        at_tiles[t] = at

    def emit_pv(t):
        j, g = groups[t]
        if g == 0:
            out_ps_by_j[j] = [
                ps_sm.tile([128, 512], f32, name=f"out_ps{j}_{s}", tag="ps")
                for s in range(N_SQSUB)
            ]
        at = at_tiles.pop(t)
        for c in range(GRP):
            ch = g * GRP + c
            for sq in range(N_SQSUB):
                nc.tensor.matmul(
                    out_ps_by_j[j][sq][:, : A + 2],
                    lhsT=at[:, c, bass.ts(sq, SQSUB)],
                    rhs=v_sb[ch],
                    start=(ch == 0),
                    stop=(ch == N_SKCH - 1),
                )

    def emit_evac(j):
        # raw [num | den] rows to HBM in bf16 via ONE packed tile + one DMA
        # per j-block (host does the divide). Copies split across ACT and
        # DVE so neither queue stalls the next group.
        ob = opool.tile([128, N_SQSUB * (A + 2)], bf16)
        for sq in range(N_SQSUB):
            src = out_ps_by_j[j][sq][:, : A + 2]
            dst = ob[:, bass.ts(sq, A + 2)]
            if sq < 2:
                nc.scalar.copy(dst, src)
            else:
                nc.vector.tensor_copy(dst, src)
        nc.sync.dma_start(out=out[j], in_=ob)

    n = len(groups)
    # last two score groups are hoisted ahead of their PV batches (deeper
    # lookahead at the stream tail) so the final exp+mask latency hides
    # under the remaining PV work instead of draining exposed
    order = []
    for t in range(n - 2):
        order.append(("sc", t))
        if t >= LA:
            order.append(("pv", t - LA))
    order += [("sc", n - 2), ("sc", n - 1)]
    order += [("pv", t) for t in range(n - 2 - LA, n)]
    for kind, t in order:
        if kind == "sc":
            emit_scores(t)
        else:
            emit_pv(t)
            jd, gd = groups[t]
            if gd == N_GRP - 1:
                emit_evac(jd)


def build_nc():
    from contextlib import ExitStack

    import concourse.bacc as bacc
    import concourse.tile as tile
    import concourse.mybir as mybir

    f32 = mybir.dt.float32
    f16 = mybir.dt.float16
    bf16 = mybir.dt.bfloat16

    nc = bacc.Bacc("TRN2", target_bir_lowering=False, debug=False)
    xT = nc.dram_tensor("xT", [2, 128, 2, 1024], f16, kind="ExternalInput").ap()
    maskT = nc.dram_tensor(
        "maskT", [N_SQBLK, 128, N_SKCH, SQBLK], f16, kind="ExternalInput"
    ).ap()
    W_pack = nc.dram_tensor("W_pack", [128, 2, WCOL], f16, kind="ExternalInput").ap()
    bias_pack = nc.dram_tensor("bias_pack", [128, 4], f32, kind="ExternalInput").ap()
    row_pack = nc.dram_tensor(
        "row_pack", [1, A + 2 + 128], f16, kind="ExternalInput"
    ).ap()
    out = nc.dram_tensor(
        "out", [N_SQBLK, 128, N_SQSUB * (A + 2)], bf16, kind="ExternalOutput"
    ).ap()

    tensors = (xT, maskT, W_pack, bias_pack, row_pack, out)
    with tile.TileContext(nc) as tc:
        with ExitStack() as ctx:
            _emit(nc, tc, ctx, tensors)
    nc.compile()
    return nc


def pack_inputs(x, mask, Wq, bq, Wk, bk, Wv, bv):
    """Host-side packing: per-core input maps (core c <- batch c)."""
    hdt = np.float16
    x = np.asarray(x, dtype=np.float32)
    mask = np.asarray(mask)
    # maskT[b, j, p, c, s] = mask[b, j*512+s, c*128+p], as {0.0, 1.0}
    from concurrent.futures import ThreadPoolExecutor

    def _pack_mask(b):
        return np.ascontiguousarray(
            mask[b]
            .transpose(1, 0)
            .reshape(N_SKCH, 128, N_SQBLK, SQBLK)
            .transpose(2, 1, 0, 3)
            .astype(hdt)
        )

    with ThreadPoolExecutor(max_workers=8) as tp:
        mt = list(tp.map(_pack_mask, range(B)))
    # W_pack[p, e, :] = [wq | wk | wv+pad] for E-chunk e
    Wq = np.asarray(Wq, hdt).reshape(2, 128, A)
    Wk = np.asarray(Wk, hdt).reshape(2, 128, A)
    Wv = np.concatenate([np.asarray(Wv, hdt), np.zeros((E, 2), hdt)], axis=1).reshape(
        2, 128, A + 2
    )
    W_pack = np.ascontiguousarray(
        np.concatenate([Wq, Wk, Wv], axis=2).transpose(1, 0, 2)
    )
    bq = np.asarray(bq, np.float32).reshape(2, 128)
    bk = np.asarray(bk, np.float32).reshape(2, 128)
    bias_pack = np.ascontiguousarray(
        np.stack([bq[0], bq[1], bk[0], bk[1]], axis=1)
    )
    row_pack = np.concatenate(
        [
            np.asarray(bv, hdt).reshape(-1),
            np.ones(2, hdt),
            np.ones(128, hdt),
        ]
    ).reshape(1, A + 2 + 128)
    in_maps = []
    for b in range(N_CORES):
        # xT[half, p, e, s_in_half] from x[b] [S, E]
        xb = np.ascontiguousarray(
            x[b].T.astype(hdt).reshape(2, 128, 2, 1024).transpose(2, 1, 0, 3)
        )
        in_maps.append(
            {
                "xT": xb,
                "maskT": mt[b],
                "W_pack": W_pack,
                "bias_pack": bias_pack,
                "row_pack": row_pack,
            }
        )
    return in_maps


_NC_CACHE = None


def _get_nc():
    global _NC_CACHE
    if _NC_CACHE is None:
        _NC_CACHE = build_nc()
    return _NC_CACHE


def unpack_out(raw):
    """[B, 4, 128, 4*(A+2)] bf16 raw [num|den] tiles -> [B, S, A] f32."""
    raw = np.asarray(raw).astype(np.float32)
    raw = raw.reshape(B, N_SQBLK, 128, N_SQSUB, A + 2).transpose(0, 1, 3, 2, 4)
    raw = raw.reshape(B, S, A + 2)
    return raw[:, :, :A] / raw[:, :, A : A + 1]


def kernel(x, mask, Wq, bq, Wk, bk, Wv, bv):
    from concourse.bass_utils import run_bass_kernel_spmd

    in_maps = pack_inputs(x, mask, Wq, bq, Wk, bk, Wv, bv)
    nc = _get_nc()
    res = run_bass_kernel_spmd(nc, in_maps, core_ids=list(range(N_CORES)))
    raw = np.stack(
        [np.asarray(res.results[c]["out"]) for c in range(N_CORES)], axis=0
    )
    return unpack_out(raw)


if __name__ == "__main__":
    nc = build_nc()
    n = sum(len(bb.instructions) for bb in nc.main_func.blocks)
    print("built ok; instructions:", n)


# revision 25
# speedup vs baseline: 1.1920x; 1.0521x over previous
"""Masked attention for (B=8, S=2048, E=A=256), f32 in/out.

Sharding: data-parallel over batch B across the 8 NeuronCores (one batch
element per core, no collectives).

Per-core dataflow (all on-chip after the input DMAs):
  xT[E,S] -> q8,k8 ([128, a-chunk=2, S] fp8e4; bias added on evacuation)
          -> v [S, A+2] fp16 (bias via K=1 ones-row matmul; cols A,A+1 hold
             1.0 so the PV matmul also produces the softmax denominator)
  scores TRANSPOSED, fp8 DoubleRow (contraction A=256 in one matmul):
    scT[sk_chunk=128p, sq 512] = k8_chunk.T @ q8  (psum tile [128, 1024])
  attnT = exp(scT/16) * maskT  (no max subtraction: |scores/16| < ~3)
  outP[sq=128p, A+2] += attnT_chunk.T @ v_chunk   (accumulate over sk, fp16)
  out rows = [num | den] in bf16; the final num/den divide runs on host.

Schedule notes:
 - junk matmuls (no data deps) run from the first post-preamble cycle so the
   PE HAM clock gate reaches 8/8 before the projections arrive.
 - x arrives as ONE big DMA per half (DMA trigger instructions cost ~0.65us
   of queue time each, so many small DMAs starve the PE at the head); all
   weights ride one packed tensor; mask DMAs are gated behind x so they do
   not steal HBM bandwidth from the critical-path loads.
 - the attention loop is software-pipelined one group ahead (scores for
   group t+1 are queued before the PV matmuls of group t) so the exp+mask
   latency stays hidden across j-block boundaries.
"""

import sys

sys.path.insert(0, "/opt/trn_rl_repo")

import numpy as np
import ml_dtypes

B, S, E, A = 8, 2048, 256, 256
N_CORES = 8

SQBLK = 512                 # Sq rows per outer block
N_SQBLK = S // SQBLK        # 4
SQSUB = 128                 # Sq rows per PV psum tile
N_SQSUB = SQBLK // SQSUB    # 4
SKCH = 128                  # Sk rows per score chunk (psum partitions)
N_SKCH = S // SKCH          # 16
GRP = 2                     # sk chunks per scores psum tile ([128, GRP*SQBLK])
N_GRP = N_SKCH // GRP       # 8
MTILE = 4                   # sk chunks per mask sbuf tile
N_WARM = 16                 # junk warm-up matmuls (N=256 cold ~215ns each)
WCOL = 2 * A + (A + 2)      # packed weight columns: wq | wk | wv+ones-pad

SCALE = 1.0 / np.sqrt(np.float32(A))


def _emit(nc, tc, ctx, tensors):
    import concourse.bass as bass
    import concourse.mybir as mybir

    f32 = mybir.dt.float32
    f16 = mybir.dt.float16
    bf16 = mybir.dt.bfloat16
    f8 = mybir.dt.float8e4
    AF = mybir.ActivationFunctionType
    DR = mybir.MatmulPerfMode.DoubleRow

    xT, maskT, W_pack, bias_pack, row_pack, out = tensors

    consts = ctx.enter_context(tc.tile_pool(name="consts", bufs=1))
    big = ctx.enter_context(tc.tile_pool(name="big", bufs=1))
    mpool = ctx.enter_context(tc.tile_pool(name="mask", bufs=16))
    epool = ctx.enter_context(tc.tile_pool(name="exp", bufs=4))
    apool = ctx.enter_context(tc.tile_pool(name="attn", bufs=4))
    opool = ctx.enter_context(tc.tile_pool(name="outsb", bufs=6))
    spool = ctx.enter_context(tc.tile_pool(name="small", bufs=8))
    ps_sc = ctx.enter_context(tc.tile_pool(name="ps_sc", bufs=2, space="PSUM"))
    ps_sm = ctx.enter_context(tc.tile_pool(name="ps_sm", bufs=4, space="PSUM"))

    # ---- HAM warm-up: dense junk matmul stream (results never consumed) so
    # the PE clock gate is already 8/8 when the projections arrive ----
    warm_sb = consts.tile([128, 256], f16, tag="warm_sb")
    nc.gpsimd.memset(warm_sb, 1.0)
    warm_ps = ps_sc.tile([128, GRP * SQBLK], f32, name="warm_ps", tag="sc")
    for _ in range(N_WARM):
        nc.tensor.matmul(
            warm_ps[:, :256], lhsT=warm_sb[:, :128], rhs=warm_sb, start=True, stop=True
        )

    # ---- inputs: weights first on the scalar HWDGE ring (their ring-FIFO
    # position guarantees they are not starved by the mask traffic), one big
    # DMA per x half, tiny bias/row DMAs ----
    wpack_sb = consts.tile([128, 2, WCOL], f16, tag="wpack")
    nc.scalar.dma_start(out=wpack_sb, in_=W_pack)
    xh_sb = []
    for h in range(2):
        t = big.tile([128, 2, 1024], f16, name=f"xh{h}", tag=f"xh{h}")
        (nc.sync if h == 0 else nc.scalar).dma_start(out=t, in_=xT[h])
        xh_sb.append(t)
    Wq_sb = [wpack_sb[:, e, 0:A] for e in range(2)]
    Wk_sb = [wpack_sb[:, e, A : 2 * A] for e in range(2)]
    Wv_sb = [wpack_sb[:, e, 2 * A : WCOL] for e in range(2)]

    bias_sb = consts.tile([128, 4], f32, tag="bias_pack")
    nc.sync.dma_start(out=bias_sb, in_=bias_pack)
    bq_sb = [bias_sb[:, 0:1], bias_sb[:, 1:2]]
    bk_sb = [bias_sb[:, 2:3], bias_sb[:, 3:4]]
    row_sb = consts.tile([1, A + 2 + 128], f16, tag="row_pack")
    nc.sync.dma_start(out=row_sb, in_=row_pack)
    bv_sb = row_sb[:, : A + 2]
    ones_sb = row_sb[:, A + 2 :]

    def x_rhs(e, j):  # [128, 512] moving operand for the qk projections
        return xh_sb[j // 2][:, e, bass.ts(j % 2, SQBLK)]

    def x_lhsT(e, c):  # [128, 128] stationary operand for the v projection
        j = c // 4
        return xh_sb[j // 2][:, e, bass.ds((j % 2) * SQBLK + (c % 4) * 128, 128)]

    # ---- mask DMAs: gpsimd ring. A short serial busy-chain occupies the
    # gpsimd queue first, delaying the mask DMA triggers ~3us so the 8MB of
    # mask traffic does not steal HBM bandwidth from the critical-path x/W
    # loads (the queue is drained in scheduled order; the chain is ready at
    # t=0 and sits ahead of the triggers) ----
    dly0 = spool.tile([128, 2048], f16, tag="dly0")
    dly1 = spool.tile([128, 2048], f16, tag="dly1")
    nc.gpsimd.memset(dly0, 0.0)
    nc.gpsimd.tensor_copy(dly1, dly0)
    nc.gpsimd.tensor_copy(dly0, dly1)
    mask_tiles = {}
    for j in range(N_SQBLK):
        for t in range(N_SKCH // MTILE):
            mt = mpool.tile([128, MTILE, SQBLK], f16, name=f"mask{j}_{t}", tag="mask")
            nc.gpsimd.dma_start(out=mt, in_=maskT[j][:, bass.ts(t, MTILE), :])
            mask_tiles[(j, t)] = mt

    # ---- projections ----
    # q8/k8: [128, a-chunk, S] fp8e4 (DoubleRow layout: partition = a%128,
    # middle dim = a-chunk, free = sequence). psum per (a, Sq512), E-chunk
    # accumulated; bias added + fp8 quantized on the DVE evacuation.
    q8_sb = big.tile([128, 2, S], f8, tag="q8")
    k8_sb = big.tile([128, 2, S], f8, tag="k8")
    v_sb = [None] * N_SKCH
    qk_steps = [
        (jp, a, wi)
        for jp in ((0, 1), (2, 3))
        for a in range(2)
        for wi in range(2)
    ]
    for s, (jp, a, wi) in enumerate(qk_steps):
        W_sb, b_sb, dst = (
            (Wq_sb, bq_sb[a], q8_sb),
            (Wk_sb, bk_sb[a], k8_sb),
        )[wi]
        pss = [
            ps_sm.tile([128, 512], f32, name=f"pp{s}_{j}", tag="ps") for j in jp
        ]
        for e in range(2):
            for i, j in enumerate(jp):
                nc.tensor.matmul(
                    pss[i],
                    lhsT=W_sb[e][:, bass.ts(a, 128)],
                    rhs=x_rhs(e, j),
                    start=(e == 0),
                    stop=(e == 1),
                )
        cpair = (2 * s, 2 * s + 1)
        vps = [
            ps_sm.tile([128, 512], f32, name=f"vp{c}", tag="ps") for c in cpair
        ]
        for e in range(2):
            for i, c in enumerate(cpair):
                nc.tensor.matmul(
                    vps[i][:, : A + 2],
                    lhsT=x_lhsT(e, c),
                    rhs=Wv_sb[e],
                    start=(e == 0),
                    stop=False,
                )
        for i, c in enumerate(cpair):
            nc.tensor.matmul(
                vps[i][:, : A + 2],
                lhsT=ones_sb,
                rhs=bv_sb,
                start=False,
                stop=True,
            )
        for i, j in enumerate(jp):
            nc.vector.tensor_scalar_add(dst[:, a, bass.ts(j, SQBLK)], pss[i], b_sb)
        for i, c in enumerate(cpair):
            vt = big.tile([128, A + 2], f16, tag=f"v{c}", name=f"v{c}")
            nc.scalar.copy(vt, vps[i][:, : A + 2])
            v_sb[c] = vt

    # ---- attention: flat pipeline over (j, g) groups, scores one group
    # ahead of the PV matmuls so exp+mask latency hides at j boundaries ----
    groups = [(j, g) for j in range(N_SQBLK) for g in range(N_GRP)]
    LA = 1
    at_tiles = {}
    out_ps_by_j = {}

    def emit_scores(t):
        j, g = groups[t]
        js = bass.ts(j, SQBLK)
        sc = ps_sc.tile([128, GRP * SQBLK], f32, tag="sc")
        for c in range(GRP):
            ch = g * GRP + c
            nc.tensor.matmul(
                sc[:, bass.ts(c, SQBLK)],
                lhsT=k8_sb[:, :, bass.ts(ch, 128)],
                rhs=q8_sb[:, :, js],
                start=True,
                stop=True,
                perf_mode=DR,
            )
        ex = epool.tile([128, GRP * SQBLK], f16)
        nc.scalar.activation(ex, sc, AF.Exp, bias=0.0, scale=float(SCALE))
        at = apool.tile([128, GRP, SQBLK], f16)
        mslice = mask_tiles[(j, (g * GRP) // MTILE)][
            :, bass.ds((g * GRP) % MTILE, GRP), :
        ]
        nc.vector.tensor_mul(at, ex.rearrange("p (c s) -> p c s", c=GRP), mslice)
        at_tiles[t] = at

    def emit_pv(t):
        j, g = groups[t]
        if g == 0:
            out_ps_by_j[j] = [
                ps_sm.tile([128, 512], f32, name=f"out_ps{j}_{s}", tag="ps")
                for s in range(N_SQSUB)
            ]
        at = at_tiles.pop(t)
        for c in range(GRP):
            ch = g * GRP + c
            for sq in range(N_SQSUB):
                nc.tensor.matmul(
                    out_ps_by_j[j][sq][:, : A + 2],
                    lhsT=at[:, c, bass.ts(sq, SQSUB)],
                    rhs=v_sb[ch],
                    start=(ch == 0),
                    stop=(ch == N_SKCH - 1),
                )

    def emit_evac(j):
        # raw [num | den] rows to HBM in bf16 via ONE packed tile + one DMA
        # per j-block (host does the divide). Copies split across ACT and
        # DVE so neither queue stalls the next group.
        ob = opool.tile([128, N_SQSUB * (A + 2)], bf16)
        for sq in range(N_SQSUB):
            src = out_ps_by_j[j][sq][:, : A + 2]
            dst = ob[:, bass.ts(sq, A + 2)]
            if sq < 2:
                nc.scalar.copy(dst, src)
            else:
                nc.vector.tensor_copy(dst, src)
        nc.sync.dma_start(out=out[j], in_=ob)

    n = len(groups)
    # last two score groups are hoisted ahead of their PV batches (deeper
    # lookahead at the stream tail) so the final exp+mask latency hides
    # under the remaining PV work instead of draining exposed
    order = []
    for t in range(n - 2):
        order.append(("sc", t))
        if t >= LA:
            order.append(("pv", t - LA))
    order += [("sc", n - 2), ("sc", n - 1)]
    order += [("pv", t) for t in range(n - 2 - LA, n)]
    for kind, t in order:
        if kind == "sc":
            emit_scores(t)
        else:
            emit_pv(t)
            jd, gd = groups[t]
            if gd == N_GRP - 1:
                emit_evac(jd)


def build_nc():
    from contextlib import ExitStack

    import concourse.bacc as bacc
    import concourse.tile as tile
    import concourse.mybir as mybir

    f32 = mybir.dt.float32
    f16 = mybir.dt.float16
    bf16 = mybir.dt.bfloat16

    nc = bacc.Bacc("TRN2", target_bir_lowering=False, debug=False)
    xT = nc.dram_tensor("xT", [2, 128, 2, 1024], f16, kind="ExternalInput").ap()
    maskT = nc.dram_tensor(
        "maskT", [N_SQBLK, 128, N_SKCH, SQBLK], f16, kind="ExternalInput"
    ).ap()
    W_pack = nc.dram_tensor("W_pack", [128, 2, WCOL], f16, kind="ExternalInput").ap()
    bias_pack = nc.dram_tensor("bias_pack", [128, 4], f32, kind="ExternalInput").ap()
    row_pack = nc.dram_tensor(
        "row_pack", [1, A + 2 + 128], f16, kind="ExternalInput"
    ).ap()
    out = nc.dram_tensor(
        "out", [N_SQBLK, 128, N_SQSUB * (A + 2)], bf16, kind="ExternalOutput"
    ).ap()

    tensors = (xT, maskT, W_pack, bias_pack, row_pack, out)
    with tile.TileContext(nc) as tc:
        with ExitStack() as ctx:
            _emit(nc, tc, ctx, tensors)
    nc.compile()
    return nc


def pack_inputs(x, mask, Wq, bq, Wk, bk, Wv, bv):
    """Host-side packing: per-core input maps (core c <- batch c)."""
    hdt = np.float16
    x = np.asarray(x, dtype=np.float32)
    mask = np.asarray(mask)
    # maskT[b, j, p, c, s] = mask[b, j*512+s, c*128+p], as {0.0, 1.0}
    from concurrent.futures import ThreadPoolExecutor

    def _pack_mask(b):
        return np.ascontiguousarray(
            mask[b]
            .transpose(1, 0)
            .reshape(N_SKCH, 128, N_SQBLK, SQBLK)
            .transpose(2, 1, 0, 3)
            .astype(hdt)
        )

    with ThreadPoolExecutor(max_workers=8) as tp:
        mt = list(tp.map(_pack_mask, range(B)))
    # W_pack[p, e, :] = [wq | wk | wv+pad] for E-chunk e
    Wq = np.asarray(Wq, hdt).reshape(2, 128, A)
    Wk = np.asarray(Wk, hdt).reshape(2, 128, A)
    Wv = np.concatenate([np.asarray(Wv, hdt), np.zeros((E, 2), hdt)], axis=1).reshape(
        2, 128, A + 2
    )
    W_pack = np.ascontiguousarray(
        np.concatenate([Wq, Wk, Wv], axis=2).transpose(1, 0, 2)
    )
    bq = np.asarray(bq, np.float32).reshape(2, 128)
    bk = np.asarray(bk, np.float32).reshape(2, 128)
    bias_pack = np.ascontiguousarray(
        np.stack([bq[0], bq[1], bk[0], bk[1]], axis=1)
    )
    row_pack = np.concatenate(
        [
            np.asarray(bv, hdt).reshape(-1),
            np.ones(2, hdt),
            np.ones(128, hdt),
        ]
    ).reshape(1, A + 2 + 128)
    in_maps = []
    for b in range(N_CORES):
        # xT[half, p, e, s_in_half] from x[b] [S, E]
        xb = np.ascontiguousarray(
            x[b].T.astype(hdt).reshape(2, 128, 2, 1024).transpose(2, 1, 0, 3)
        )
        in_maps.append(
            {
                "xT": xb,
                "maskT": mt[b],
                "W_pack": W_pack,
                "bias_pack": bias_pack,
                "row_pack": row_pack,
            }
        )
    return in_maps


_NC_CACHE = None


def _get_nc():
    global _NC_CACHE
    if _NC_CACHE is None:
        _NC_CACHE = build_nc()
    return _NC_CACHE


def unpack_out(raw):
    """[B, 4, 128, 4*(A+2)] bf16 raw [num|den] tiles -> [B, S, A] f32."""
    raw = np.asarray(raw).astype(np.float32)
    raw = raw.reshape(B, N_SQBLK, 128, N_SQSUB, A + 2).transpose(0, 1, 3, 2, 4)
    raw = raw.reshape(B, S, A + 2)
    return raw[:, :, :A] / raw[:, :, A : A + 1]


def kernel(x, mask, Wq, bq, Wk, bk, Wv, bv):
    from concourse.bass_utils import run_bass_kernel_spmd

    in_maps = pack_inputs(x, mask, Wq, bq, Wk, bk, Wv, bv)
    nc = _get_nc()
    res = run_bass_kernel_spmd(nc, in_maps, core_ids=list(range(N_CORES)))
    raw = np.stack(
        [np.asarray(res.results[c]["out"]) for c in range(N_CORES)], axis=0
    )
    return unpack_out(raw)


if __name__ == "__main__":
    nc = build_nc()
    n = sum(len(bb.instructions) for bb in nc.main_func.blocks)
    print("built ok; instructions:", n)


# revision 28
# speedup vs baseline: 1.2080x; 1.0134x over previous
"""Masked attention for (B=8, S=2048, E=A=256), f32 in/out.

Sharding: data-parallel over batch B across the 8 NeuronCores (one batch
element per core, no collectives).

Per-core dataflow (all on-chip after the input DMAs):
  xT[E,S] -> q8,k8 ([128, a-chunk=2, S] fp8e4; bias added on evacuation)
          -> v [S, A+2] fp16 (bias via K=1 ones-row matmul; cols A,A+1 hold
             1.0 so the PV matmul also produces the softmax denominator)
  scores TRANSPOSED, fp8 DoubleRow (contraction A=256 in one matmul):
    scT[sk_chunk=128p, sq 512] = k8_chunk.T @ q8  (psum tile [128, 1024])
  attnT = exp(scT/16) * maskT  (no max subtraction: |scores/16| < ~3)
  outP[sq=128p, A+2] += attnT_chunk.T @ v_chunk   (accumulate over sk, fp16)
  out rows = [num | den] in bf16; the final num/den divide runs on host.

Schedule notes:
 - junk matmuls (no data deps) run from the first post-preamble cycle so the
   PE HAM clock gate reaches 8/8 before the projections arrive.
 - x arrives as ONE big DMA per half (DMA trigger instructions cost ~0.65us
   of queue time each, so many small DMAs starve the PE at the head); all
   weights ride one packed tensor; mask DMAs are gated behind x so they do
   not steal HBM bandwidth from the critical-path loads.
 - the attention loop is software-pipelined one group ahead (scores for
   group t+1 are queued before the PV matmuls of group t) so the exp+mask
   latency stays hidden across j-block boundaries.
"""

import sys

sys.path.insert(0, "/opt/trn_rl_repo")

import numpy as np
import ml_dtypes

B, S, E, A = 8, 2048, 256, 256
N_CORES = 8

SQBLK = 512                 # Sq rows per outer block
N_SQBLK = S // SQBLK        # 4
SQSUB = 128                 # Sq rows per PV psum tile
N_SQSUB = SQBLK // SQSUB    # 4
SKCH = 128                  # Sk rows per score chunk (psum partitions)
N_SKCH = S // SKCH          # 16
GRP = 2                     # sk chunks per scores psum tile ([128, GRP*SQBLK])
N_GRP = N_SKCH // GRP       # 8
MTILE = 4                   # sk chunks per mask sbuf tile
N_WARM = 20                 # junk warm-up matmuls (N=256 cold ~215ns each)
WCOL = 2 * A + (A + 2)      # packed weight columns: wq | wk | wv+ones-pad

SCALE = 1.0 / np.sqrt(np.float32(A))


def _emit(nc, tc, ctx, tensors):
    import concourse.bass as bass
    import concourse.mybir as mybir

    f32 = mybir.dt.float32
    f16 = mybir.dt.float16
    bf16 = mybir.dt.bfloat16
    f8 = mybir.dt.float8e4
    AF = mybir.ActivationFunctionType
    DR = mybir.MatmulPerfMode.DoubleRow

    xT, maskT, W_pack, bias_pack, row_pack, out = tensors

    consts = ctx.enter_context(tc.tile_pool(name="consts", bufs=1))
    big = ctx.enter_context(tc.tile_pool(name="big", bufs=1))
    mpool = ctx.enter_context(tc.tile_pool(name="mask", bufs=16))
    epool = ctx.enter_context(tc.tile_pool(name="exp", bufs=4))
    apool = ctx.enter_context(tc.tile_pool(name="attn", bufs=4))
    opool = ctx.enter_context(tc.tile_pool(name="outsb", bufs=6))
    spool = ctx.enter_context(tc.tile_pool(name="small", bufs=8))
    ps_sc = ctx.enter_context(tc.tile_pool(name="ps_sc", bufs=2, space="PSUM"))
    ps_sm = ctx.enter_context(tc.tile_pool(name="ps_sm", bufs=4, space="PSUM"))

    # ---- HAM warm-up: dense junk matmul stream (results never consumed) so
    # the PE clock gate is already 8/8 when the projections arrive ----
    warm_sb = consts.tile([128, 256], f16, tag="warm_sb")
    nc.gpsimd.memset(warm_sb, 1.0)
    warm_ps = ps_sc.tile([128, GRP * SQBLK], f32, name="warm_ps", tag="sc")
    for _ in range(N_WARM):
        nc.tensor.matmul(
            warm_ps[:, :256], lhsT=warm_sb[:, :128], rhs=warm_sb, start=True, stop=True
        )

    # ---- inputs: weights first on the scalar HWDGE ring (their ring-FIFO
    # position guarantees they are not starved by the mask traffic), one big
    # DMA per x half, tiny bias/row DMAs ----
    wpack_sb = consts.tile([128, 2, WCOL], f16, tag="wpack")
    nc.scalar.dma_start(out=wpack_sb, in_=W_pack)
    xh_sb = []
    for h in range(2):
        t = big.tile([128, 2, 1024], f16, name=f"xh{h}", tag=f"xh{h}")
        (nc.sync if h == 0 else nc.scalar).dma_start(out=t, in_=xT[h])
        xh_sb.append(t)
    Wq_sb = [wpack_sb[:, e, 0:A] for e in range(2)]
    Wk_sb = [wpack_sb[:, e, A : 2 * A] for e in range(2)]
    Wv_sb = [wpack_sb[:, e, 2 * A : WCOL] for e in range(2)]

    bias_sb = consts.tile([128, 4], f32, tag="bias_pack")
    nc.sync.dma_start(out=bias_sb, in_=bias_pack)
    bq_sb = [bias_sb[:, 0:1], bias_sb[:, 1:2]]
    bk_sb = [bias_sb[:, 2:3], bias_sb[:, 3:4]]
    row_sb = consts.tile([1, A + 2 + 128], f16, tag="row_pack")
    nc.sync.dma_start(out=row_sb, in_=row_pack)
    bv_sb = row_sb[:, : A + 2]
    ones_sb = row_sb[:, A + 2 :]

    def x_rhs(e, j):  # [128, 512] moving operand for the qk projections
        return xh_sb[j // 2][:, e, bass.ts(j % 2, SQBLK)]

    def x_lhsT(e, c):  # [128, 128] stationary operand for the v projection
        j = c // 4
        return xh_sb[j // 2][:, e, bass.ds((j % 2) * SQBLK + (c % 4) * 128, 128)]

    # ---- mask DMAs: gpsimd ring. A short serial busy-chain occupies the
    # gpsimd queue first, delaying the mask DMA triggers ~3us so the 8MB of
    # mask traffic does not steal HBM bandwidth from the critical-path x/W
    # loads (the queue is drained in scheduled order; the chain is ready at
    # t=0 and sits ahead of the triggers) ----
    dly0 = spool.tile([128, 2048], f16, tag="dly0")
    dly1 = spool.tile([128, 2048], f16, tag="dly1")
    nc.gpsimd.memset(dly0, 0.0)
    nc.gpsimd.tensor_copy(dly1, dly0)
    nc.gpsimd.tensor_copy(dly0, dly1)
    mask_tiles = {}
    for j in range(N_SQBLK):
        for t in range(N_SKCH // MTILE):
            mt = mpool.tile([128, MTILE, SQBLK], f16, name=f"mask{j}_{t}", tag="mask")
            nc.gpsimd.dma_start(out=mt, in_=maskT[j][:, bass.ts(t, MTILE), :])
            mask_tiles[(j, t)] = mt

    # ---- projections ----
    # q8/k8: [128, a-chunk, S] fp8e4 (DoubleRow layout: partition = a%128,
    # middle dim = a-chunk, free = sequence). psum per (a, Sq512), E-chunk
    # accumulated; bias added + fp8 quantized on the DVE evacuation.
    q8_sb = big.tile([128, 2, S], f8, tag="q8")
    k8_sb = big.tile([128, 2, S], f8, tag="k8")
    v_sb = [None] * N_SKCH
    qk_steps = [
        (jp, a, wi)
        for jp in ((0, 1), (2, 3))
        for a in range(2)
        for wi in range(2)
    ]
    for s, (jp, a, wi) in enumerate(qk_steps):
        W_sb, b_sb, dst = (
            (Wq_sb, bq_sb[a], q8_sb),
            (Wk_sb, bk_sb[a], k8_sb),
        )[wi]
        pss = [
            ps_sm.tile([128, 512], f32, name=f"pp{s}_{j}", tag="ps") for j in jp
        ]
        for e in range(2):
            for i, j in enumerate(jp):
                nc.tensor.matmul(
                    pss[i],
                    lhsT=W_sb[e][:, bass.ts(a, 128)],
                    rhs=x_rhs(e, j),
                    start=(e == 0),
                    stop=(e == 1),
                )
        cpair = (2 * s, 2 * s + 1)
        vps = [
            ps_sm.tile([128, 512], f32, name=f"vp{c}", tag="ps") for c in cpair
        ]
        for e in range(2):
            for i, c in enumerate(cpair):
                nc.tensor.matmul(
                    vps[i][:, : A + 2],
                    lhsT=x_lhsT(e, c),
                    rhs=Wv_sb[e],
                    start=(e == 0),
                    stop=False,
                )
        for i, c in enumerate(cpair):
            nc.tensor.matmul(
                vps[i][:, : A + 2],
                lhsT=ones_sb,
                rhs=bv_sb,
                start=False,
                stop=True,
            )
        for i, j in enumerate(jp):
            nc.vector.tensor_scalar_add(dst[:, a, bass.ts(j, SQBLK)], pss[i], b_sb)
        for i, c in enumerate(cpair):
            vt = big.tile([128, A + 2], f16, tag=f"v{c}", name=f"v{c}")
            nc.scalar.copy(vt, vps[i][:, : A + 2])
            v_sb[c] = vt

    # ---- attention: flat pipeline over (j, g) groups, scores one group
    # ahead of the PV matmuls so exp+mask latency hides at j boundaries ----
    groups = [(j, g) for j in range(N_SQBLK) for g in range(N_GRP)]
    LA = 1
    at_tiles = {}
    out_ps_by_j = {}

    def emit_scores(t):
        j, g = groups[t]
        js = bass.ts(j, SQBLK)
        sc = ps_sc.tile([128, GRP * SQBLK], f32, tag="sc")
        for c in range(GRP):
            ch = g * GRP + c
            nc.tensor.matmul(
                sc[:, bass.ts(c, SQBLK)],
                lhsT=k8_sb[:, :, bass.ts(ch, 128)],
                rhs=q8_sb[:, :, js],
                start=True,
                stop=True,
                perf_mode=DR,
            )
        ex = epool.tile([128, GRP * SQBLK], f16)
        nc.scalar.activation(ex, sc, AF.Exp, bias=0.0, scale=float(SCALE))
        at = apool.tile([128, GRP * SQBLK], f16)
        # flat 2D multiply: the mask slice is contiguous across its chunk
        # pair, so DVE sees a plain [128, 1024] step-1 op (2x mode)
        mslice = mask_tiles[(j, (g * GRP) // MTILE)][
            :, bass.ds((g * GRP) % MTILE, GRP), :
        ].rearrange("p c s -> p (c s)")
        nc.vector.tensor_mul(at, ex, mslice)
        at_tiles[t] = at.rearrange("p (c s) -> p c s", c=GRP)

    def emit_pv(t):
        j, g = groups[t]
        if g == 0:
            out_ps_by_j[j] = [
                ps_sm.tile([128, 512], f32, name=f"out_ps{j}_{s}", tag="ps")
                for s in range(N_SQSUB)
            ]
        at = at_tiles.pop(t)
        for c in range(GRP):
            ch = g * GRP + c
            for sq in range(N_SQSUB):
                nc.tensor.matmul(
                    out_ps_by_j[j][sq][:, : A + 2],
                    lhsT=at[:, c, bass.ts(sq, SQSUB)],
                    rhs=v_sb[ch],
                    start=(ch == 0),
                    stop=(ch == N_SKCH - 1),
                )

    def emit_evac(j):
        # raw [num | den] rows to HBM in bf16 via ONE packed tile + one DMA
        # per j-block (host does the divide). Copies split across ACT and
        # DVE so neither queue stalls the next group. The last j-block DMAs
        # per-sq instead so the transfers overlap the remaining copies.
        ob = opool.tile([128, N_SQSUB * (A + 2)], bf16)
        last = j == N_SQBLK - 1
        for sq in range(N_SQSUB):
            src = out_ps_by_j[j][sq][:, : A + 2]
            dst = ob[:, bass.ts(sq, A + 2)]
            if sq < 2:
                nc.scalar.copy(dst, src)
            else:
                nc.vector.tensor_copy(dst, src)
            if last:
                nc.sync.dma_start(out=out[j][:, bass.ts(sq, A + 2)], in_=dst)
        if not last:
            nc.sync.dma_start(out=out[j], in_=ob)

    n = len(groups)
    # last two score groups are hoisted ahead of their PV batches (deeper
    # lookahead at the stream tail) so the final exp+mask latency hides
    # under the remaining PV work instead of draining exposed
    order = []
    for t in range(n - 2):
        order.append(("sc", t))
        if t >= LA:
            order.append(("pv", t - LA))
    order += [("sc", n - 2), ("sc", n - 1)]
    order += [("pv", t) for t in range(n - 2 - LA, n)]
    for kind, t in order:
        if kind == "sc":
            emit_scores(t)
        else:
            emit_pv(t)
            jd, gd = groups[t]
            if gd == N_GRP - 1:
                emit_evac(jd)


def build_nc():
    from contextlib import ExitStack

    import concourse.bacc as bacc
    import concourse.tile as tile
    import concourse.mybir as mybir

    f32 = mybir.dt.float32
    f16 = mybir.dt.float16
    bf16 = mybir.dt.bfloat16

    nc = bacc.Bacc("TRN2", target_bir_lowering=False, debug=False)
    xT = nc.dram_tensor("xT", [2, 128, 2, 1024], f16, kind="ExternalInput").ap()
    maskT = nc.dram_tensor(
        "maskT", [N_SQBLK, 128, N_SKCH, SQBLK], f16, kind="ExternalInput"
    ).ap()
    W_pack = nc.dram_tensor("W_pack", [128, 2, WCOL], f16, kind="ExternalInput").ap()
    bias_pack = nc.dram_tensor("bias_pack", [128, 4], f32, kind="ExternalInput").ap()
    row_pack = nc.dram_tensor(
        "row_pack", [1, A + 2 + 128], f16, kind="ExternalInput"
    ).ap()
    out = nc.dram_tensor(
        "out", [N_SQBLK, 128, N_SQSUB * (A + 2)], bf16, kind="ExternalOutput"
    ).ap()

    tensors = (xT, maskT, W_pack, bias_pack, row_pack, out)
    with tile.TileContext(nc) as tc:
        with ExitStack() as ctx:
            _emit(nc, tc, ctx, tensors)
    nc.compile()
    return nc


def pack_inputs(x, mask, Wq, bq, Wk, bk, Wv, bv):
    """Host-side packing: per-core input maps (core c <- batch c)."""
    hdt = np.float16
    x = np.asarray(x, dtype=np.float32)
    mask = np.asarray(mask)
    # maskT[b, j, p, c, s] = mask[b, j*512+s, c*128+p], as {0.0, 1.0}
    from concurrent.futures import ThreadPoolExecutor

    def _pack_mask(b):
        return np.ascontiguousarray(
            mask[b]
            .transpose(1, 0)
            .reshape(N_SKCH, 128, N_SQBLK, SQBLK)
            .transpose(2, 1, 0, 3)
            .astype(hdt)
        )

    with ThreadPoolExecutor(max_workers=8) as tp:
        mt = list(tp.map(_pack_mask, range(B)))
    # W_pack[p, e, :] = [wq | wk | wv+pad] for E-chunk e
    Wq = np.asarray(Wq, hdt).reshape(2, 128, A)
    Wk = np.asarray(Wk, hdt).reshape(2, 128, A)
    Wv = np.concatenate([np.asarray(Wv, hdt), np.zeros((E, 2), hdt)], axis=1).reshape(
        2, 128, A + 2
    )
    W_pack = np.ascontiguousarray(
        np.concatenate([Wq, Wk, Wv], axis=2).transpose(1, 0, 2)
    )
    bq = np.asarray(bq, np.float32).reshape(2, 128)
    bk = np.asarray(bk, np.float32).reshape(2, 128)
    bias_pack = np.ascontiguousarray(
        np.stack([bq[0], bq[1], bk[0], bk[1]], axis=1)
    )
    row_pack = np.concatenate(
        [
            np.asarray(bv, hdt).reshape(-1),
            np.ones(2, hdt),
            np.ones(128, hdt),
        ]
    ).reshape(1, A + 2 + 128)
    in_maps = []
    for b in range(N_CORES):
        # xT[half, p, e, s_in_half] from x[b] [S, E]
        xb = np.ascontiguousarray(
            x[b].T.astype(hdt).reshape(2, 128, 2, 1024).transpose(2, 1, 0, 3)
        )
        in_maps.append(
            {
                "xT": xb,
                "maskT": mt[b],
                "W_pack": W_pack,
                "bias_pack": bias_pack,
                "row_pack": row_pack,
            }
        )
    return in_maps


_NC_CACHE = None


def _get_nc():
    global _NC_CACHE
    if _NC_CACHE is None:
        _NC_CACHE = build_nc()
    return _NC_CACHE


def unpack_out(raw):
    """[B, 4, 128, 4*(A+2)] bf16 raw [num|den] tiles -> [B, S, A] f32."""
    raw = np.asarray(raw).astype(np.float32)
    raw = raw.reshape(B, N_SQBLK, 128, N_SQSUB, A + 2).transpose(0, 1, 3, 2, 4)
    raw = raw.reshape(B, S, A + 2)
    return raw[:, :, :A] / raw[:, :, A : A + 1]


def kernel(x, mask, Wq, bq, Wk, bk, Wv, bv):
    from concourse.bass_utils import run_bass_kernel_spmd

    in_maps = pack_inputs(x, mask, Wq, bq, Wk, bk, Wv, bv)
    nc = _get_nc()
    res = run_bass_kernel_spmd(nc, in_maps, core_ids=list(range(N_CORES)))
    raw = np.stack(
        [np.asarray(res.results[c]["out"]) for c in range(N_CORES)], axis=0
    )
    return unpack_out(raw)


if __name__ == "__main__":
    nc = build_nc()
    n = sum(len(bb.instructions) for bb in nc.main_func.blocks)
    print("built ok; instructions:", n)


# revision 29
# speedup vs baseline: 1.2151x; 1.0059x over previous
"""Masked attention for (B=8, S=2048, E=A=256), f32 in/out.

Sharding: data-parallel over batch B across the 8 NeuronCores (one batch
element per core, no collectives).

Per-core dataflow (all on-chip after the input DMAs):
  xT[E,S] -> q8,k8 ([128, a-chunk=2, S] fp8e4; bias added on evacuation)
          -> v [S, A+2] fp16 (bias via K=1 ones-row matmul; cols A,A+1 hold
             1.0 so the PV matmul also produces the softmax denominator)
  scores TRANSPOSED, fp8 DoubleRow (contraction A=256 in one matmul):
    scT[sk_chunk=128p, sq 512] = k8_chunk.T @ q8  (psum tile [128, 1024])
  attnT = exp(scT/16) * maskT  (no max subtraction: |scores/16| < ~3)
  outP[sq=128p, A+2] += attnT_chunk.T @ v_chunk   (accumulate over sk, fp16)
  out rows = [num | den] in bf16; the final num/den divide runs on host.

Schedule notes:
 - junk matmuls (no data deps) run from the first post-preamble cycle so the
   PE HAM clock gate reaches 8/8 before the projections arrive.
 - x arrives as ONE big DMA per half (DMA trigger instructions cost ~0.65us
   of queue time each, so many small DMAs starve the PE at the head); all
   weights ride one packed tensor; mask DMAs are gated behind x so they do
   not steal HBM bandwidth from the critical-path loads.
 - the attention loop is software-pipelined one group ahead (scores for
   group t+1 are queued before the PV matmuls of group t) so the exp+mask
   latency stays hidden across j-block boundaries.
"""

import sys

sys.path.insert(0, "/opt/trn_rl_repo")

import numpy as np
import ml_dtypes

B, S, E, A = 8, 2048, 256, 256
N_CORES = 8

SQBLK = 512                 # Sq rows per outer block
N_SQBLK = S // SQBLK        # 4
SQSUB = 128                 # Sq rows per PV psum tile
N_SQSUB = SQBLK // SQSUB    # 4
SKCH = 128                  # Sk rows per score chunk (psum partitions)
N_SKCH = S // SKCH          # 16
GRP = 2                     # sk chunks per scores psum tile ([128, GRP*SQBLK])
N_GRP = N_SKCH // GRP       # 8
MTILE = 4                   # sk chunks per mask sbuf tile
N_WARM = 20                 # junk warm-up matmuls (N=256 cold ~215ns each)
WCOL = 2 * A + (A + 2)      # packed weight columns: wq | wk | wv+ones-pad

SCALE = 1.0 / np.sqrt(np.float32(A))


def _emit(nc, tc, ctx, tensors):
    import concourse.bass as bass
    import concourse.mybir as mybir

    f32 = mybir.dt.float32
    f16 = mybir.dt.float16
    bf16 = mybir.dt.bfloat16
    f8 = mybir.dt.float8e4
    AF = mybir.ActivationFunctionType
    DR = mybir.MatmulPerfMode.DoubleRow

    xT, maskT, W_pack, bias_pack, row_pack, out = tensors

    consts = ctx.enter_context(tc.tile_pool(name="consts", bufs=1))
    big = ctx.enter_context(tc.tile_pool(name="big", bufs=1))
    mpool = ctx.enter_context(tc.tile_pool(name="mask", bufs=16))
    epool = ctx.enter_context(tc.tile_pool(name="exp", bufs=4))
    apool = ctx.enter_context(tc.tile_pool(name="attn", bufs=4))
    opool = ctx.enter_context(tc.tile_pool(name="outsb", bufs=6))
    spool = ctx.enter_context(tc.tile_pool(name="small", bufs=8))
    ps_sc = ctx.enter_context(tc.tile_pool(name="ps_sc", bufs=2, space="PSUM"))
    ps_sm = ctx.enter_context(tc.tile_pool(name="ps_sm", bufs=4, space="PSUM"))

    # ---- HAM warm-up: dense junk matmul stream (results never consumed) so
    # the PE clock gate is already 8/8 when the projections arrive ----
    warm_sb = consts.tile([128, 256], f16, tag="warm_sb")
    nc.gpsimd.memset(warm_sb, 1.0)
    warm_ps = ps_sc.tile([128, GRP * SQBLK], f32, name="warm_ps", tag="sc")
    for _ in range(N_WARM):
        nc.tensor.matmul(
            warm_ps[:, :256], lhsT=warm_sb[:, :128], rhs=warm_sb, start=True, stop=True
        )

    # ---- inputs: weights first on the scalar HWDGE ring (their ring-FIFO
    # position guarantees they are not starved by the mask traffic), one big
    # DMA per x half, tiny bias/row DMAs ----
    wpack_sb = consts.tile([128, 2, WCOL], f16, tag="wpack")
    nc.scalar.dma_start(out=wpack_sb, in_=W_pack)
    xh_sb = []
    for h in range(2):
        t = big.tile([128, 2, 1024], f16, name=f"xh{h}", tag=f"xh{h}")
        (nc.sync if h == 0 else nc.scalar).dma_start(out=t, in_=xT[h])
        xh_sb.append(t)
    Wq_sb = [wpack_sb[:, e, 0:A] for e in range(2)]
    Wk_sb = [wpack_sb[:, e, A : 2 * A] for e in range(2)]
    Wv_sb = [wpack_sb[:, e, 2 * A : WCOL] for e in range(2)]

    bias_sb = consts.tile([128, 4], f32, tag="bias_pack")
    nc.sync.dma_start(out=bias_sb, in_=bias_pack)
    bq_sb = [bias_sb[:, 0:1], bias_sb[:, 1:2]]
    bk_sb = [bias_sb[:, 2:3], bias_sb[:, 3:4]]
    row_sb = consts.tile([1, A + 2 + 128], f16, tag="row_pack")
    nc.sync.dma_start(out=row_sb, in_=row_pack)
    bv_sb = row_sb[:, : A + 2]
    ones_sb = row_sb[:, A + 2 :]

    def x_rhs(e, j):  # [128, 512] moving operand for the qk projections
        return xh_sb[j // 2][:, e, bass.ts(j % 2, SQBLK)]

    def x_lhsT(e, c):  # [128, 128] stationary operand for the v projection
        j = c // 4
        return xh_sb[j // 2][:, e, bass.ds((j % 2) * SQBLK + (c % 4) * 128, 128)]

    # ---- mask DMAs: gpsimd ring. A short serial busy-chain occupies the
    # gpsimd queue first, delaying the mask DMA triggers ~3us so the 8MB of
    # mask traffic does not steal HBM bandwidth from the critical-path x/W
    # loads (the queue is drained in scheduled order; the chain is ready at
    # t=0 and sits ahead of the triggers) ----
    dly0 = spool.tile([128, 2048], f16, tag="dly0")
    dly1 = spool.tile([128, 2048], f16, tag="dly1")
    nc.gpsimd.memset(dly0, 0.0)
    nc.gpsimd.tensor_copy(dly1, dly0)
    nc.gpsimd.tensor_copy(dly0, dly1)
    mask_tiles = {}
    for j in range(N_SQBLK):
        for t in range(N_SKCH // MTILE):
            mt = mpool.tile([128, MTILE, SQBLK], f16, name=f"mask{j}_{t}", tag="mask")
            nc.gpsimd.dma_start(out=mt, in_=maskT[j][:, bass.ts(t, MTILE), :])
            mask_tiles[(j, t)] = mt

    # ---- projections ----
    # q8/k8: [128, a-chunk, S] fp8e4 (DoubleRow layout: partition = a%128,
    # middle dim = a-chunk, free = sequence). psum per (a, Sq512), E-chunk
    # accumulated; bias added + fp8 quantized on the DVE evacuation.
    q8_sb = big.tile([128, 2, S], f8, tag="q8")
    k8_sb = big.tile([128, 2, S], f8, tag="k8")
    v_sb = [None] * N_SKCH
    qk_steps = [
        (jp, a, wi)
        for jp in ((0, 1), (2, 3))
        for a in range(2)
        for wi in range(2)
    ]
    for s, (jp, a, wi) in enumerate(qk_steps):
        W_sb, b_sb, dst = (
            (Wq_sb, bq_sb[a], q8_sb),
            (Wk_sb, bk_sb[a], k8_sb),
        )[wi]
        pss = [
            ps_sm.tile([128, 512], f32, name=f"pp{s}_{j}", tag="ps") for j in jp
        ]
        for e in range(2):
            for i, j in enumerate(jp):
                nc.tensor.matmul(
                    pss[i],
                    lhsT=W_sb[e][:, bass.ts(a, 128)],
                    rhs=x_rhs(e, j),
                    start=(e == 0),
                    stop=(e == 1),
                )
        cpair = (2 * s, 2 * s + 1)
        vps = [
            ps_sm.tile([128, 512], f32, name=f"vp{c}", tag="ps") for c in cpair
        ]
        for e in range(2):
            for i, c in enumerate(cpair):
                nc.tensor.matmul(
                    vps[i][:, : A + 2],
                    lhsT=x_lhsT(e, c),
                    rhs=Wv_sb[e],
                    start=(e == 0),
                    stop=False,
                )
        for i, c in enumerate(cpair):
            nc.tensor.matmul(
                vps[i][:, : A + 2],
                lhsT=ones_sb,
                rhs=bv_sb,
                start=False,
                stop=True,
            )
        for i, j in enumerate(jp):
            nc.vector.tensor_scalar_add(dst[:, a, bass.ts(j, SQBLK)], pss[i], b_sb)
        for i, c in enumerate(cpair):
            vt = big.tile([128, A + 2], f16, tag=f"v{c}", name=f"v{c}")
            nc.scalar.copy(vt, vps[i][:, : A + 2])
            v_sb[c] = vt

    # ---- attention: flat pipeline over (j, g) groups, scores one group
    # ahead of the PV matmuls so exp+mask latency hides at j boundaries ----
    groups = [(j, g) for j in range(N_SQBLK) for g in range(N_GRP)]
    LA = 1
    at_tiles = {}
    out_ps_by_j = {}

    def emit_scores(t):
        j, g = groups[t]
        js = bass.ts(j, SQBLK)
        sc = ps_sc.tile([128, GRP * SQBLK], f32, tag="sc")
        for c in range(GRP):
            ch = g * GRP + c
            nc.tensor.matmul(
                sc[:, bass.ts(c, SQBLK)],
                lhsT=k8_sb[:, :, bass.ts(ch, 128)],
                rhs=q8_sb[:, :, js],
                start=True,
                stop=True,
                perf_mode=DR,
            )
        ex = epool.tile([128, GRP * SQBLK], f16)
        nc.scalar.activation(ex, sc, AF.Exp, bias=0.0, scale=float(SCALE))
        at = apool.tile([128, GRP * SQBLK], f16)
        # flat 2D multiply: the mask slice is contiguous across its chunk
        # pair, so DVE sees a plain [128, 1024] step-1 op (2x mode)
        mslice = mask_tiles[(j, (g * GRP) // MTILE)][
            :, bass.ds((g * GRP) % MTILE, GRP), :
        ].rearrange("p c s -> p (c s)")
        nc.vector.tensor_mul(at, ex, mslice)
        at_tiles[t] = at.rearrange("p (c s) -> p c s", c=GRP)

    def emit_pv(t):
        j, g = groups[t]
        if g == 0:
            out_ps_by_j[j] = [
                ps_sm.tile([128, 512], f32, name=f"out_ps{j}_{s}", tag="ps")
                for s in range(N_SQSUB)
            ]
        at = at_tiles.pop(t)
        for c in range(GRP):
            ch = g * GRP + c
            for sq in range(N_SQSUB):
                nc.tensor.matmul(
                    out_ps_by_j[j][sq][:, : A + 2],
                    lhsT=at[:, c, bass.ts(sq, SQSUB)],
                    rhs=v_sb[ch],
                    start=(ch == 0),
                    stop=(ch == N_SKCH - 1),
                )

    def emit_evac(j):
        # raw [num | den] rows to HBM in bf16 via ONE packed tile + one DMA
        # per j-block (host does the divide). Copies split across ACT and
        # DVE so neither queue stalls the next group. The last j-block DMAs
        # per-sq instead so the transfers overlap the remaining copies.
        ob = opool.tile([128, N_SQSUB * (A + 2)], bf16)
        last = j == N_SQBLK - 1
        for sq in range(N_SQSUB):
            src = out_ps_by_j[j][sq][:, : A + 2]
            dst = ob[:, bass.ts(sq, A + 2)]
            if sq < 2:
                nc.scalar.copy(dst, src)
            else:
                nc.vector.tensor_copy(dst, src)
            if last:
                ring = nc.sync if sq % 2 == 0 else nc.scalar
                ring.dma_start(out=out[j][:, bass.ts(sq, A + 2)], in_=dst)
        if not last:
            nc.sync.dma_start(out=out[j], in_=ob)

    n = len(groups)
    # last two score groups are hoisted ahead of their PV batches (deeper
    # lookahead at the stream tail) so the final exp+mask latency hides
    # under the remaining PV work instead of draining exposed
    order = []
    for t in range(n - 2):
        order.append(("sc", t))
        if t >= LA:
            order.append(("pv", t - LA))
    order += [("sc", n - 2), ("sc", n - 1)]
    order += [("pv", t) for t in range(n - 2 - LA, n)]
    for kind, t in order:
        if kind == "sc":
            emit_scores(t)
        else:
            emit_pv(t)
            jd, gd = groups[t]
            if gd == N_GRP - 1:
                emit_evac(jd)


def build_nc():
    from contextlib import ExitStack

    import concourse.bacc as bacc
    import concourse.tile as tile
    import concourse.mybir as mybir

    f32 = mybir.dt.float32
    f16 = mybir.dt.float16
    bf16 = mybir.dt.bfloat16

    nc = bacc.Bacc("TRN2", target_bir_lowering=False, debug=False)
    xT = nc.dram_tensor("xT", [2, 128, 2, 1024], f16, kind="ExternalInput").ap()
    maskT = nc.dram_tensor(
        "maskT", [N_SQBLK, 128, N_SKCH, SQBLK], f16, kind="ExternalInput"
    ).ap()
    W_pack = nc.dram_tensor("W_pack", [128, 2, WCOL], f16, kind="ExternalInput").ap()
    bias_pack = nc.dram_tensor("bias_pack", [128, 4], f32, kind="ExternalInput").ap()
    row_pack = nc.dram_tensor(
        "row_pack", [1, A + 2 + 128], f16, kind="ExternalInput"
    ).ap()
    out = nc.dram_tensor(
        "out", [N_SQBLK, 128, N_SQSUB * (A + 2)], bf16, kind="ExternalOutput"
    ).ap()

    tensors = (xT, maskT, W_pack, bias_pack, row_pack, out)
    with tile.TileContext(nc) as tc:
        with ExitStack() as ctx:
            _emit(nc, tc, ctx, tensors)
    nc.compile()
    return nc


def pack_inputs(x, mask, Wq, bq, Wk, bk, Wv, bv):
    """Host-side packing: per-core input maps (core c <- batch c)."""
    hdt = np.float16
    x = np.asarray(x, dtype=np.float32)
    mask = np.asarray(mask)
    # maskT[b, j, p, c, s] = mask[b, j*512+s, c*128+p], as {0.0, 1.0}
    from concurrent.futures import ThreadPoolExecutor

    def _pack_mask(b):
        return np.ascontiguousarray(
            mask[b]
            .transpose(1, 0)
            .reshape(N_SKCH, 128, N_SQBLK, SQBLK)
            .transpose(2, 1, 0, 3)
            .astype(hdt)
        )

    with ThreadPoolExecutor(max_workers=8) as tp:
        mt = list(tp.map(_pack_mask, range(B)))
    # W_pack[p, e, :] = [wq | wk | wv+pad] for E-chunk e
    Wq = np.asarray(Wq, hdt).reshape(2, 128, A)
    Wk = np.asarray(Wk, hdt).reshape(2, 128, A)
    Wv = np.concatenate([np.asarray(Wv, hdt), np.zeros((E, 2), hdt)], axis=1).reshape(
        2, 128, A + 2
    )
    W_pack = np.ascontiguousarray(
        np.concatenate([Wq, Wk, Wv], axis=2).transpose(1, 0, 2)
    )
    bq = np.asarray(bq, np.float32).reshape(2, 128)
    bk = np.asarray(bk, np.float32).reshape(2, 128)
    bias_pack = np.ascontiguousarray(
        np.stack([bq[0], bq[1], bk[0], bk[1]], axis=1)
    )
    row_pack = np.concatenate(
        [
            np.asarray(bv, hdt).reshape(-1),
            np.ones(2, hdt),
            np.ones(128, hdt),
        ]
    ).reshape(1, A + 2 + 128)
    in_maps = []
    for b in range(N_CORES):
        # xT[half, p, e, s_in_half] from x[b] [S, E]
        xb = np.ascontiguousarray(
            x[b].T.astype(hdt).reshape(2, 128, 2, 1024).transpose(2, 1, 0, 3)
        )
        in_maps.append(
            {
                "xT": xb,
                "maskT": mt[b],
                "W_pack": W_pack,
                "bias_pack": bias_pack,
                "row_pack": row_pack,
            }
        )
    return in_maps


_NC_CACHE = None


def _get_nc():
    global _NC_CACHE
    if _NC_CACHE is None:
        _NC_CACHE = build_nc()
    return _NC_CACHE


def unpack_out(raw):
    """[B, 4, 128, 4*(A+2)] bf16 raw [num|den] tiles -> [B, S, A] f32."""
    raw = np.asarray(raw).astype(np.float32)
    raw = raw.reshape(B, N_SQBLK, 128, N_SQSUB, A + 2).transpose(0, 1, 3, 2, 4)
    raw = raw.reshape(B, S, A + 2)
    return raw[:, :, :A] / raw[:, :, A : A + 1]


def kernel(x, mask, Wq, bq, Wk, bk, Wv, bv):
    from concourse.bass_utils import run_bass_kernel_spmd

    in_maps = pack_inputs(x, mask, Wq, bq, Wk, bk, Wv, bv)
    nc = _get_nc()
    res = run_bass_kernel_spmd(nc, in_maps, core_ids=list(range(N_CORES)))
    raw = np.stack(
        [np.asarray(res.results[c]["out"]) for c in range(N_CORES)], axis=0
    )
    return unpack_out(raw)


if __name__ == "__main__":
    nc = build_nc()
    n = sum(len(bb.instructions) for bb in nc.main_func.blocks)
    print("built ok; instructions:", n)
